# revision 15
# baseline (speedup 1.0000x reference)
"""Multi-head attention (2 batches x 4 heads, n=4096, dh=128) on 8 trn2 cores.

Sharding: one (batch, head) pair per NeuronCore (batch*heads = 8 = n_cores).

Per core pipeline:
  phase 1: host ships x^T fp16 (no on-chip x transposes); qkv projection via
    fp16 matmuls; q^T,k^T kept fp16 resident; V transposed on PE then stored
    as fp8e4 V8 plus an fp8 residual V8r (error compensation for the fp8 PV
    matmuls).
  phase 2 (per 512-query group): S^T = K_c^T Q chunks (fp16 matmul, fp32
    PSUM) -> exp via ACT directly to fp8e4 P^T (scale=1/sqrt(dh), bias=-2
    folded), optionally some chunk-pairs via a Schraudolph-style exp2 on DVE
    (single fused tensor_scalar writing int8 bits == fp8e4 bit pattern) ->
    P^T (V8 + V8r) accumulated in PSUM with fp8 DoubleRow matmuls (2 k-tiles
    per instruction, 0.5 cyc/row) -> softmax denominator via fp8 DoubleRow
    ones-matmuls producing [128q, 1] partials directly (no DVE add tree) ->
    reciprocal, PE transpose, scale, DMA out.
"""

import numpy as np
from contextlib import ExitStack

import concourse.bass as bass
import concourse.mybir as mybir
import concourse.tile as tile
from concourse.bass_utils import run_bass_kernel_spmd
from concourse.masks import make_identity
from bass_rust import ScopedClock

F32 = mybir.dt.float32
F32R = mybir.dt.float32r
F16 = mybir.dt.float16
F8 = mybir.dt.float8e4
I8 = mybir.dt.int8
AF = mybir.ActivationFunctionType
ALU = mybir.AluOpType
DR = mybir.MatmulPerfMode.DoubleRow

B = 2
HEADS = 4
N = 4096
DIM = 512
DH = 128
NCORES = 8

SCALE = DH ** -0.5        # folded into the exp activation
EXP_BIAS = -2.0           # exp(s*SCALE - 2): keeps P in [~0.04, ~0.5]
LOG2E = float(np.log2(np.e))

NG = 8                    # query groups of 512
QG = 512                  # queries per group
KC = 32                   # key chunks of 128
NSP = 16                  # chunk pairs (spans) per group

# Schraudolph exp2-via-bits constants: fp8e4(2^t) bits ~= round(8*(t+7)+C)
SCH_A = SCALE * LOG2E * 8.0
SCH_C = -0.32             # spread-centering correction (calibrated on CPU)
SCH_B = (7.0 + EXP_BIAS * LOG2E) * 8.0 + SCH_C

# which spans (chunk pairs) run Schraudolph-exp on DVE instead of ACT exp
SCH_SPANS = ()

RESID = False             # fp8 V residual-compensation second PV pass

MAXW = 1  # max sync waits this walrus build accepts per instruction


class _TC(tile.TileContext):
    """TileContext with a post-pass that splits instructions' sem waits
    across preceding same-engine NOPs: this container's walrus rejects any
    instruction carrying more than MAXW sync waits (CoreV3 setupSyncWait:
    "Too many sync wait commands")."""

    def _drain_and_barrier(self, tick_clock, wait_clock):
        nc = self.nc
        drain_inst = nc.sync.drain()
        wait_clock.add_sem_waits(
            drain_inst.ins, ScopedClock({None: tick_clock.global_clock})
        )
        nc.all_engine_barrier()
        assert self.sems is not None
        popped = nc._tile_sem_poison_stack.pop()
        assert popped is self._sem_poison
        nc.clear_and_free_semaphores(list(self.sems.allocated().values()))
        nc.all_engine_barrier()
        self._split_excess_waits()

    def _split_excess_waits(self):
        nc = self.nc
        cur_insts = nn_bb_insts(nc)
        for bb in nc.m.functions[0].blocks:
            insts = bb.instructions
            pos = 0
            while pos < len(insts):
                inst = insts[pos]
                si = inst.sync_info
                waits = list(si.on_wait) if si and si.on_wait else []
                if len(waits) <= MAXW:
                    pos += 1
                    continue
                si.on_wait = waits[-MAXW:]
                rest = waits[:-MAXW]
                eng = nc.engines[inst.engine]
                for i in range(0, len(rest), MAXW):
                    chunk = rest[i : i + MAXW]
                    nop = eng.nop()
                    # relocate the freshly appended nop from cur_bb's tail
                    # to just before the offending instruction
                    popped = cur_insts.pop()
                    assert popped.name == nop.ins.name
                    nsi = nop.ins.sync_info
                    if nsi is None:
                        nop.ins.sync_info = mybir.SyncInfo(
                            on_wait=chunk, on_update=[]
                        )
                    else:
                        nsi.on_wait = chunk
                    insts.insert(pos, nop.ins)
                    pos += 1
                pos += 1


def nn_bb_insts(nc):
    bb = nc.cur_bb
    assert bb is not None
    return bb.bb.instructions


def build(repeat=1, skip=(), loop_reps=None):
    nc = bass.Bass()
    xt = nc.dram_tensor("xt", [DIM, N], F16, kind="ExternalInput")
    # per-head W^T, columns [q | k | v], each [DIM, DH]
    wt = nc.dram_tensor("wt", [DIM, 3 * DH], F16, kind="ExternalInput")
    bqkv = nc.dram_tensor("bqkv", [3, DH], F32, kind="ExternalInput")
    y = nc.dram_tensor("y", [N, DH], F32, kind="ExternalOutput")
    dnd = nc.dram_tensor("dnd", [NG, 512], F32, kind="Internal")

    with ExitStack() as ctx:
        tc = ctx.enter_context(_TC(nc))

        singles = ctx.enter_context(tc.tile_pool(name="singles", bufs=1))

        identsrc = singles.tile([128, 128], F32)
        make_identity(nc, identsrc)
        ident = singles.tile([128, 128], F32R)
        nc.vector.tensor_copy(out=ident, in_=identsrc)
        ident16 = singles.tile([128, 128], F16)
        nc.vector.tensor_copy(out=ident16, in_=identsrc)
        ones8 = singles.tile([128, 2, 128], F8)
        nc.vector.memset(ones8, 1.0)
        expb = singles.tile([128, 1], F32)
        nc.vector.memset(expb, EXP_BIAS)

        # weights [dm-within-chunk, dm-chunk, 3*dh] and biases [dh, 3]
        wt_sb = singles.tile([128, 4, 3 * DH], F16)
        nc.sync.dma_start(out=wt_sb, in_=wt[:, :].rearrange("(c p) o -> p c o", p=128))
        b_sb = singles.tile([128, 3], F32)
        nc.sync.dma_start(out=b_sb, in_=bqkv[:, :].rearrange("t d -> d t"))

        # resident activations
        qd = singles.tile([128, N], F16)             # Q^T  [dh, n]
        kd = singles.tile([128, N], F16)             # K^T  [dh, n]
        v8 = singles.tile([128, KC, DH], F8)         # V    [n-in-chunk, chunk, dh]
        v8r = singles.tile([128, KC, DH], F8)        # fp8 residual of V

        if loop_reps is None:
            for _rep in range(repeat):
                _body(nc, tc, ident, ident16, ones8, expb, wt_sb, b_sb,
                      qd, kd, v8, v8r, xt, dnd, y, skip)
        else:
            with tc.For_i(0, loop_reps, 1):
                _body(nc, tc, ident, ident16, ones8, expb, wt_sb, b_sb,
                      qd, kd, v8, v8r, xt, dnd, y, skip)

    return nc


def _body(nc, tc, ident, ident16, ones8, expb, wt_sb, b_sb, qd, kd,
          v8, v8r, xt, dnd, y, skip=()):
    # ---------------- phase 1: qkv projection ----------------
    ph1 = ExitStack()
    xin = ph1.enter_context(tc.tile_pool(name="xin", bufs=2))
    vtmp = ph1.enter_context(tc.tile_pool(name="vtmp", bufs=2))
    v16 = ph1.enter_context(tc.tile_pool(name="v16", bufs=2))
    ps_mm = ph1.enter_context(tc.tile_pool(name="ps_mm", bufs=3, space="PSUM"))
    ps_v = ph1.enter_context(tc.tile_pool(name="ps_v", bufs=4, space="PSUM"))

    # resident x^T, one whole-tensor DMA (prefetches under the previous
    # iteration's phase 2)
    xt_t = xin.tile([128, 4, N], F16)            # x^T [dm-part, dm-chunk, n]
    if "ph1" not in skip:
        nc.sync.dma_start(
            out=xt_t, in_=xt[:, :].rearrange("(c p) n -> p c n", p=128)
        )

    for nch in range(8) if "ph1" not in skip else []:  # 512-token chunks
        n_sl = slice(nch * 512, (nch + 1) * 512)
        for m in range(3):                       # q, k, v
            pm = ps_mm.tile([128, 512], F32)
            for d in range(4):
                nc.tensor.matmul(
                    pm,
                    lhsT=wt_sb[:, d, m * DH : (m + 1) * DH],
                    rhs=xt_t[:, d, n_sl],
                    start=(d == 0),
                    stop=(d == 3),
                )
            if m == 0:
                nc.vector.tensor_scalar_add(qd[:, n_sl], pm, b_sb[:, 0:1])
            elif m == 1:
                nc.vector.tensor_scalar_add(kd[:, n_sl], pm, b_sb[:, 1:2])
            else:
                vt = vtmp.tile([128, 512], F16)
                nc.vector.tensor_scalar_add(vt, pm, b_sb[:, 2:3])
                for j in range(4):
                    tv = ps_v.tile([128, 128], F16)
                    nc.tensor.transpose(
                        tv, vt[:, j * 128 : (j + 1) * 128], ident16
                    )
                    kc = nch * 4 + j
                    # fp8 V (and fp8 residual V - fp8(V) when RESID)
                    nc.scalar.copy(v8[:, kc, :], tv)
                    if RESID:
                        vu = v16.tile([128, 128], F16, tag="vu")
                        nc.vector.tensor_copy(out=vu, in_=v8[:, kc, :])
                        nc.vector.tensor_tensor(
                            out=v8r[:, kc, :], in0=tv, in1=vu, op=ALU.subtract
                        )

    ph1.close()

    # ---------------- phase 2: attention ----------------
    ph2 = ExitStack()
    pt_pool = ph2.enter_context(tc.tile_pool(name="pt", bufs=2))
    ot_pool = ph2.enter_context(tc.tile_pool(name="ot", bufs=2))
    ob_pool = ph2.enter_context(tc.tile_pool(name="ob", bufs=2))
    rc_pool = ph2.enter_context(tc.tile_pool(name="rc", bufs=2))
    ps_st = ph2.enter_context(tc.tile_pool(name="ps_st", bufs=2, space="PSUM"))
    ps_pv = ph2.enter_context(tc.tile_pool(name="ps_pv", bufs=1, space="PSUM"))
    ps_dn = ph2.enter_context(tc.tile_pool(name="ps_dn", bufs=1, space="PSUM"))
    ps_sm = ph2.enter_context(tc.tile_pool(name="ps_sm", bufs=1, space="PSUM"))

    for g in range(NG) if "attn" not in skip else []:
        q_sl = slice(g * QG, (g + 1) * QG)
        # P^T for the whole group, fp8, [key-in-chunk, chunk, query]
        pt8 = pt_pool.tile([128, KC, QG], F8, tag="pt8")
        pt8i = pt8.bitcast(I8)

        for sp in range(NSP):
            stp = ps_st.tile([128, 2, 512], F32)
            for j in range(2):
                kc = 2 * sp + j
                nc.tensor.matmul(
                    stp[:, j, :],
                    lhsT=kd[:, kc * 128 : (kc + 1) * 128],
                    rhs=qd[:, q_sl],
                    start=True,
                    stop=True,
                )
            if "exp" in skip:
                pass
            elif sp in SCH_SPANS and "sch" not in skip:
                # Schraudolph: fp8e4 bits of 2^(s*SCALE*log2e + EXP_BIAS*log2e)
                nc.vector.tensor_scalar(
                    out=pt8i[:, 2 * sp : 2 * sp + 2, :],
                    in0=stp,
                    scalar1=SCH_A,
                    scalar2=SCH_B,
                    op0=ALU.mult,
                    op1=ALU.add,
                )
            else:
                nc.scalar.activation(
                    out=pt8[:, 2 * sp : 2 * sp + 2, :],
                    in_=stp,
                    func=AF.Exp,
                    scale=SCALE,
                    bias=expb,
                )

        # P^T V accumulation (fp8 DoubleRow, V8 then residual V8r)
        pv = ps_pv.tile([128, 512], F32, tag="pv")
        vvs = (v8, v8r) if (RESID and "resid" not in skip) else (v8,)
        for i, vv in enumerate(vvs) if "pv" not in skip else []:
            for c in range(NSP):
                nc.tensor.matmul(
                    pv,
                    lhsT=vv[:, 2 * c : 2 * c + 2, :],
                    rhs=pt8[:, 2 * c : 2 * c + 2, :],
                    perf_mode=DR,
                    start=(i == 0 and c == 0),
                    stop=(i == len(vvs) - 1 and c == NSP - 1),
                )

        # denominator: ones-stationary DoubleRow matmuls -> [4, 512]
        dnb = ps_dn.tile([128, 512], F32, tag="dn")
        for c in range(NSP) if "dn" not in skip else []:
            nc.tensor.matmul(
                dnb,
                lhsT=ones8,
                rhs=pt8[:, 2 * c : 2 * c + 2, :],
                perf_mode=DR,
                start=(c == 0),
                stop=(c == NSP - 1),
            )
        if "pv" in skip:
            continue
        rq = rc_pool.tile([1, 512], F32, tag="rq")
        if "dn" not in skip:
            nc.vector.reciprocal(rq, dnb[0:1, :])
        else:
            nc.vector.memset(rq, 1.0)
        # redistribute 1/dn to query-partition layout [128, 4] via a DRAM
        # round-trip (SBUF APs cannot cross partitions)
        nc.sync.dma_start(out=dnd[g : g + 1, :], in_=rq)
        rc = rc_pool.tile([128, 4], F32, tag="rc")
        nc.sync.dma_start(
            out=rc, in_=dnd[g : g + 1, :].rearrange("o (st p) -> (o p) st", p=128)
        )

        # out^T -> SBUF, then per-subtile transpose + normalize
        ot = ot_pool.tile([128, 512], F32R)
        nc.vector.tensor_copy(out=ot, in_=pv)

        tp = ps_sm.tile([128, 512], F32, tag="sm")
        for st in range(4):
            nc.tensor.transpose(
                tp[:, st * 128 : (st + 1) * 128].bitcast(F32R),
                ot[:, st * 128 : (st + 1) * 128],
                ident,
            )
        ob = ob_pool.tile([128, 4, 128], F32)
        for st in range(4):
            nc.vector.tensor_scalar_mul(
                ob[:, st, :], tp[:, st * 128 : (st + 1) * 128], rc[:, st : st + 1]
            )
        nc.sync.dma_start(
            out=y[q_sl, :].rearrange("(s p) d -> p s d", p=128), in_=ob
        )

    ph2.close()


def prep_in_maps(x, W, b):
    x = np.asarray(x, dtype=np.float32)
    W = np.asarray(W, dtype=np.float32)
    b = np.asarray(b, dtype=np.float32)
    in_maps = []
    for c in range(NCORES):
        bb, h = divmod(c, HEADS)
        rows = np.arange(DH) * HEADS + h
        wt = np.concatenate(
            [np.ascontiguousarray(W[blk * DIM + rows, :].T) for blk in range(3)],
            axis=1,
        ).astype(np.float16)  # [DIM, 3*DH]
        bs = np.stack([b[blk * DIM + rows] for blk in range(3)], axis=0)  # [3, DH]
        in_maps.append(
            {
                "xt": np.ascontiguousarray(x[bb].T).astype(np.float16),
                "wt": np.ascontiguousarray(wt),
                "bqkv": np.ascontiguousarray(bs),
            }
        )
    return in_maps


_NC = None


def kernel(x, W, b):
    global _NC
    if _NC is None:
        _NC = build()

    in_maps = prep_in_maps(x, W, b)
    res = run_bass_kernel_spmd(_NC, in_maps, core_ids=list(range(NCORES)))

    out = np.empty((B, N, HEADS * DH), dtype=np.float32)
    for c in range(NCORES):
        bb, h = divmod(c, HEADS)
        out[bb, :, h * DH : (h + 1) * DH] = res.results[c]["y"]
    return out


# revision 16
# speedup vs baseline: 1.4082x; 1.4082x over previous
"""Multi-head attention (2 batches x 4 heads, n=4096, dh=128) on 8 trn2 cores.

Sharding: one (batch, head) pair per NeuronCore (batch*heads = 8 = n_cores).

Per core pipeline:
  phase 1: host ships x^T fp16 (no on-chip x transposes); qkv projection via
    fp16 matmuls; q^T,k^T kept fp16 resident; V transposed on PE then stored
    as fp8e4 V8 plus an fp8 residual V8r (error compensation for the fp8 PV
    matmuls).
  phase 2 (per 512-query group): S^T = K_c^T Q chunks (fp16 matmul, fp32
    PSUM) -> exp via ACT directly to fp8e4 P^T (scale=1/sqrt(dh), bias=-2
    folded), optionally some chunk-pairs via a Schraudolph-style exp2 on DVE
    (single fused tensor_scalar writing int8 bits == fp8e4 bit pattern) ->
    P^T (V8 + V8r) accumulated in PSUM with fp8 DoubleRow matmuls (2 k-tiles
    per instruction, 0.5 cyc/row) -> softmax denominator via fp8 DoubleRow
    ones-matmuls producing [128q, 1] partials directly (no DVE add tree) ->
    reciprocal, PE transpose, scale, DMA out.
"""

import numpy as np
from contextlib import ExitStack

import concourse.bass as bass
import concourse.mybir as mybir
import concourse.tile as tile
from concourse.bass_utils import run_bass_kernel_spmd
from concourse.masks import make_identity
from bass_rust import ScopedClock

F32 = mybir.dt.float32
F32R = mybir.dt.float32r
F16 = mybir.dt.float16
F8 = mybir.dt.float8e4
I8 = mybir.dt.int8
AF = mybir.ActivationFunctionType
ALU = mybir.AluOpType
DR = mybir.MatmulPerfMode.DoubleRow

B = 2
HEADS = 4
N = 4096
DIM = 512
DH = 128
NCORES = 8

SCALE = DH ** -0.5        # folded into the exp activation
EXP_BIAS = -2.0           # exp(s*SCALE - 2): keeps P in [~0.04, ~0.5]
LOG2E = float(np.log2(np.e))

NG = 8                    # query groups of 512
QG = 512                  # queries per group
KC = 32                   # key chunks of 128
NSP = 16                  # chunk pairs (spans) per group

# Schraudolph exp2-via-bits constants: fp8e4(2^t) bits ~= round(8*(t+7)+C)
SCH_A = SCALE * LOG2E * 8.0
SCH_C = -0.32             # spread-centering correction (calibrated on CPU)
SCH_B = (7.0 + EXP_BIAS * LOG2E) * 8.0 + SCH_C

# which spans (chunk pairs) run Schraudolph-exp on DVE instead of ACT exp
SCH_SPANS = ()

RESID = False             # fp8 V residual-compensation second PV pass

MAXW = 1  # max sync waits this walrus build accepts per instruction


class _TC(tile.TileContext):
    """TileContext with a post-pass that splits instructions' sem waits
    across preceding same-engine NOPs: this container's walrus rejects any
    instruction carrying more than MAXW sync waits (CoreV3 setupSyncWait:
    "Too many sync wait commands")."""

    def _drain_and_barrier(self, tick_clock, wait_clock):
        nc = self.nc
        drain_inst = nc.sync.drain()
        wait_clock.add_sem_waits(
            drain_inst.ins, ScopedClock({None: tick_clock.global_clock})
        )
        nc.all_engine_barrier()
        assert self.sems is not None
        popped = nc._tile_sem_poison_stack.pop()
        assert popped is self._sem_poison
        nc.clear_and_free_semaphores(list(self.sems.allocated().values()))
        nc.all_engine_barrier()
        self._split_excess_waits()

    def _split_excess_waits(self):
        nc = self.nc
        cur_insts = nn_bb_insts(nc)
        for bb in nc.m.functions[0].blocks:
            insts = bb.instructions
            pos = 0
            while pos < len(insts):
                inst = insts[pos]
                si = inst.sync_info
                waits = list(si.on_wait) if si and si.on_wait else []
                if len(waits) <= MAXW:
                    pos += 1
                    continue
                si.on_wait = waits[-MAXW:]
                rest = waits[:-MAXW]
                eng = nc.engines[inst.engine]
                for i in range(0, len(rest), MAXW):
                    chunk = rest[i : i + MAXW]
                    nop = eng.nop()
                    # relocate the freshly appended nop from cur_bb's tail
                    # to just before the offending instruction
                    popped = cur_insts.pop()
                    assert popped.name == nop.ins.name
                    nsi = nop.ins.sync_info
                    if nsi is None:
                        nop.ins.sync_info = mybir.SyncInfo(
                            on_wait=chunk, on_update=[]
                        )
                    else:
                        nsi.on_wait = chunk
                    insts.insert(pos, nop.ins)
                    pos += 1
                pos += 1


def nn_bb_insts(nc):
    bb = nc.cur_bb
    assert bb is not None
    return bb.bb.instructions


def build(repeat=1, skip=(), loop_reps=None):
    nc = bass.Bass()
    xt = nc.dram_tensor("xt", [DIM, N], F16, kind="ExternalInput")
    # per-head W^T, columns [q | k | v], each [DIM, DH]
    wt = nc.dram_tensor("wt", [DIM, 3 * DH], F16, kind="ExternalInput")
    bqkv = nc.dram_tensor("bqkv", [3, DH], F32, kind="ExternalInput")
    y = nc.dram_tensor("y", [N, DH], F32, kind="ExternalOutput")
    dnd = nc.dram_tensor("dnd", [NG, 512], F32, kind="Internal")

    with ExitStack() as ctx:
        tc = ctx.enter_context(_TC(nc))

        singles = ctx.enter_context(tc.tile_pool(name="singles", bufs=1))

        identsrc = singles.tile([128, 128], F32)
        make_identity(nc, identsrc)
        ident = singles.tile([128, 128], F32R)
        nc.vector.tensor_copy(out=ident, in_=identsrc)
        ident16 = singles.tile([128, 128], F16)
        nc.vector.tensor_copy(out=ident16, in_=identsrc)
        ones8 = singles.tile([128, 2, 32], F8)
        nc.vector.memset(ones8, 1.0)
        expb = singles.tile([128, 1], F32)
        nc.vector.memset(expb, EXP_BIAS)

        # weights [dm-within-chunk, dm-chunk, 3*dh] and biases [dh, 3]
        wt_sb = singles.tile([128, 4, 3 * DH], F16)
        nc.sync.dma_start(out=wt_sb, in_=wt[:, :].rearrange("(c p) o -> p c o", p=128))
        b_sb = singles.tile([128, 3], F32)
        nc.sync.dma_start(out=b_sb, in_=bqkv[:, :].rearrange("t d -> d t"))

        # resident activations
        qd = singles.tile([128, N], F16)             # Q^T  [dh, n]
        kd = singles.tile([128, N], F16)             # K^T  [dh, n]
        v8 = singles.tile([128, KC, DH], F8)         # V    [n-in-chunk, chunk, dh]
        v8r = singles.tile([128, KC, DH], F8)        # fp8 residual of V

        if loop_reps is None:
            for _rep in range(repeat):
                _body(nc, tc, ident, ident16, ones8, expb, wt_sb, b_sb,
                      qd, kd, v8, v8r, xt, dnd, y, skip)
        else:
            with tc.For_i(0, loop_reps, 1):
                _body(nc, tc, ident, ident16, ones8, expb, wt_sb, b_sb,
                      qd, kd, v8, v8r, xt, dnd, y, skip)

    return nc


def _body(nc, tc, ident, ident16, ones8, expb, wt_sb, b_sb, qd, kd,
          v8, v8r, xt, dnd, y, skip=()):
    # ---------------- phase 1: qkv projection ----------------
    ph1 = ExitStack()
    xin = ph1.enter_context(tc.tile_pool(name="xin", bufs=2))
    vtmp = ph1.enter_context(tc.tile_pool(name="vtmp", bufs=2))
    v16 = ph1.enter_context(tc.tile_pool(name="v16", bufs=2))
    ps_mm = ph1.enter_context(tc.tile_pool(name="ps_mm", bufs=3, space="PSUM"))
    ps_v = ph1.enter_context(tc.tile_pool(name="ps_v", bufs=4, space="PSUM"))

    # resident x^T, one whole-tensor DMA (prefetches under the previous
    # iteration's phase 2)
    xt_t = xin.tile([128, 4, N], F16)            # x^T [dm-part, dm-chunk, n]
    if "ph1" not in skip:
        nc.sync.dma_start(
            out=xt_t, in_=xt[:, :].rearrange("(c p) n -> p c n", p=128)
        )

    for nch in range(8) if "ph1" not in skip else []:  # 512-token chunks
        n_sl = slice(nch * 512, (nch + 1) * 512)
        for m in range(3):                       # q, k, v
            pm = ps_mm.tile([128, 512], F32)
            for d in range(4):
                nc.tensor.matmul(
                    pm,
                    lhsT=wt_sb[:, d, m * DH : (m + 1) * DH],
                    rhs=xt_t[:, d, n_sl],
                    start=(d == 0),
                    stop=(d == 3),
                )
            if m == 0:
                nc.vector.tensor_scalar_add(qd[:, n_sl], pm, b_sb[:, 0:1])
            elif m == 1:
                nc.vector.tensor_scalar_add(kd[:, n_sl], pm, b_sb[:, 1:2])
            else:
                vt = vtmp.tile([128, 512], F16)
                nc.vector.tensor_scalar_add(vt, pm, b_sb[:, 2:3])
                for j in range(4):
                    tv = ps_v.tile([128, 128], F16)
                    nc.tensor.transpose(
                        tv, vt[:, j * 128 : (j + 1) * 128], ident16
                    )
                    kc = nch * 4 + j
                    # fp8 V (and fp8 residual V - fp8(V) when RESID)
                    nc.scalar.copy(v8[:, kc, :], tv)
                    if RESID:
                        vu = v16.tile([128, 128], F16, tag="vu")
                        nc.vector.tensor_copy(out=vu, in_=v8[:, kc, :])
                        nc.vector.tensor_tensor(
                            out=v8r[:, kc, :], in0=tv, in1=vu, op=ALU.subtract
                        )

    ph1.close()

    # ---------------- phase 2: attention ----------------
    ph2 = ExitStack()
    pt_pool = ph2.enter_context(tc.tile_pool(name="pt", bufs=2))
    ot_pool = ph2.enter_context(tc.tile_pool(name="ot", bufs=2))
    ob_pool = ph2.enter_context(tc.tile_pool(name="ob", bufs=2))
    rc_pool = ph2.enter_context(tc.tile_pool(name="rc", bufs=2))
    ps_st = ph2.enter_context(tc.tile_pool(name="ps_st", bufs=2, space="PSUM"))
    ps_pv = ph2.enter_context(tc.tile_pool(name="ps_pv", bufs=1, space="PSUM"))
    ps_dn = ph2.enter_context(tc.tile_pool(name="ps_dn", bufs=1, space="PSUM"))
    ps_sm = ph2.enter_context(tc.tile_pool(name="ps_sm", bufs=1, space="PSUM"))

    for g in range(NG) if "attn" not in skip else []:
        q_sl = slice(g * QG, (g + 1) * QG)
        # P^T for the whole group, fp8, [key-in-chunk, chunk, query]
        pt8 = pt_pool.tile([128, KC, QG], F8, tag="pt8")
        pt8i = pt8.bitcast(I8)

        for sp in range(NSP):
            stp = ps_st.tile([128, 2, 512], F32)
            for j in range(2):
                kc = 2 * sp + j
                nc.tensor.matmul(
                    stp[:, j, :],
                    lhsT=kd[:, kc * 128 : (kc + 1) * 128],
                    rhs=qd[:, q_sl],
                    start=True,
                    stop=True,
                )
            if "exp" in skip:
                pass
            elif sp in SCH_SPANS and "sch" not in skip:
                # Schraudolph: fp8e4 bits of 2^(s*SCALE*log2e + EXP_BIAS*log2e)
                nc.vector.tensor_scalar(
                    out=pt8i[:, 2 * sp : 2 * sp + 2, :],
                    in0=stp,
                    scalar1=SCH_A,
                    scalar2=SCH_B,
                    op0=ALU.mult,
                    op1=ALU.add,
                )
            else:
                nc.scalar.activation(
                    out=pt8[:, 2 * sp : 2 * sp + 2, :],
                    in_=stp,
                    func=AF.Exp,
                    scale=SCALE,
                    bias=expb,
                )

        # P^T V accumulation (fp8 DoubleRow, V8 then residual V8r)
        pv = ps_pv.tile([128, 512], F32, tag="pv")
        vvs = (v8, v8r) if (RESID and "resid" not in skip) else (v8,)
        for i, vv in enumerate(vvs) if "pv" not in skip else []:
            for c in range(NSP):
                nc.tensor.matmul(
                    pv,
                    lhsT=vv[:, 2 * c : 2 * c + 2, :],
                    rhs=pt8[:, 2 * c : 2 * c + 2, :],
                    perf_mode=DR,
                    start=(i == 0 and c == 0),
                    stop=(i == len(vvs) - 1 and c == NSP - 1),
                )

        # denominator: ones-stationary DoubleRow matmuls -> [4, 512]
        dnb = ps_dn.tile([32, 512], F32, tag="dn")
        for c in range(NSP) if "dn" not in skip else []:
            nc.tensor.matmul(
                dnb,
                lhsT=ones8,
                rhs=pt8[:, 2 * c : 2 * c + 2, :],
                perf_mode=DR,
                start=(c == 0),
                stop=(c == NSP - 1),
            )
        if "pv" in skip:
            continue
        rq = rc_pool.tile([1, 512], F32, tag="rq")
        if "dn" not in skip:
            nc.vector.reciprocal(rq, dnb[0:1, :])
        else:
            nc.vector.memset(rq, 1.0)
        # redistribute 1/dn to query-partition layout [128, 4] via a DRAM
        # round-trip (SBUF APs cannot cross partitions)
        nc.sync.dma_start(out=dnd[g : g + 1, :], in_=rq)
        rc = rc_pool.tile([128, 4], F32, tag="rc")
        nc.sync.dma_start(
            out=rc, in_=dnd[g : g + 1, :].rearrange("o (st p) -> (o p) st", p=128)
        )

        # out^T -> SBUF, then per-subtile transpose + normalize
        ot = ot_pool.tile([128, 512], F32R)
        nc.vector.tensor_copy(out=ot, in_=pv)

        tp = ps_sm.tile([128, 512], F32, tag="sm")
        for st in range(4):
            nc.tensor.transpose(
                tp[:, st * 128 : (st + 1) * 128].bitcast(F32R),
                ot[:, st * 128 : (st + 1) * 128],
                ident,
            )
        ob = ob_pool.tile([128, 4, 128], F32)
        for st in range(4):
            nc.vector.tensor_scalar_mul(
                ob[:, st, :], tp[:, st * 128 : (st + 1) * 128], rc[:, st : st + 1]
            )
        nc.sync.dma_start(
            out=y[q_sl, :].rearrange("(s p) d -> p s d", p=128), in_=ob
        )

    ph2.close()


def prep_in_maps(x, W, b):
    x = np.asarray(x, dtype=np.float32)
    W = np.asarray(W, dtype=np.float32)
    b = np.asarray(b, dtype=np.float32)
    in_maps = []
    for c in range(NCORES):
        bb, h = divmod(c, HEADS)
        rows = np.arange(DH) * HEADS + h
        wt = np.concatenate(
            [np.ascontiguousarray(W[blk * DIM + rows, :].T) for blk in range(3)],
            axis=1,
        ).astype(np.float16)  # [DIM, 3*DH]
        bs = np.stack([b[blk * DIM + rows] for blk in range(3)], axis=0)  # [3, DH]
        in_maps.append(
            {
                "xt": np.ascontiguousarray(x[bb].T).astype(np.float16),
                "wt": np.ascontiguousarray(wt),
                "bqkv": np.ascontiguousarray(bs),
            }
        )
    return in_maps


_NC = None


def kernel(x, W, b):
    global _NC
    if _NC is None:
        _NC = build()

    in_maps = prep_in_maps(x, W, b)
    res = run_bass_kernel_spmd(_NC, in_maps, core_ids=list(range(NCORES)))

    out = np.empty((B, N, HEADS * DH), dtype=np.float32)
    for c in range(NCORES):
        bb, h = divmod(c, HEADS)
        out[bb, :, h * DH : (h + 1) * DH] = res.results[c]["y"]
    return out
